# revision 3
# baseline (speedup 1.0000x reference)
"""Trainium2 Bass kernel for 3-layer GCN (nn_MultiLayerGCN_48773648613817).

Strategy (8 NeuronCores, SPMD):
  - Nodes sharded across cores (12500/core, padded to 12544 = 98*128).
  - Per layer:  table = dis (.) (X @ W)  computed shard-local ([node,feat] rows),
    AllGather'd into a replicated DRAM table.
  - Edges partitioned by destination core, grouped into 128-node dest windows,
    padded to 128-edge chunks (layout shared across cores; per-core data).
  - Per chunk: indirect-DMA gather of 128 source rows (one row per
    partition), one-hot selection matrix S built on DVE via is_equal against an
    iota row, TensorE matmul S^T @ msg accumulated into the window's PSUM.
  - Window epilogue: out = relu(dis * psum + b); layers 1-2 transpose back to
    X^T for the next layer's matmul, layer 3 DMAs rows to the output.
  - Message tables are fp8-e4m3 for layers 0-1 and bf16 for layer 2. The
    random per-edge gathers are DMA-byte-bound on this part, so fp8 halves
    the dominant cost for two of three layers; the last layer stays bf16 to
    keep max rel err ~0.007 (fp8 everywhere measures ~0.023 > 2e-2 gate).
    The S^T @ msg matmul runs with bf16 lhsT against fp8 rhs (PE supports
    mixed non-fp32 operand dtypes).

Self-loops are handled in the window epilogue via the hsb add (coefficient
dis^2 = 1/deg matches GCN's normalized self-loop exactly, since
msg = dis[src]*h[src] and the window epilogue multiplies by dis[dst]).
"""

import numpy as np

from concourse import bass, bacc, mybir, tile
from concourse.bass_utils import run_bass_kernel_spmd

N_NODES = 100000
N_LAYERS = 3
DIM = 128
N_CORES = 8
NSH = N_NODES // N_CORES          # 12500 real nodes per shard
P = 128
NWIN = 98                          # windows per shard
NSHP = NWIN * P                    # 12544 padded nodes per shard
N_TABLE = N_CORES * NSHP           # 100352 padded table rows

F32 = mybir.dt.float32
BF = mybir.dt.bfloat16
F8 = mybir.dt.float8e4
I32 = mybir.dt.int32

FP8_LAYERS = (True, True, False)


def _prepare(x, edge_indices, W, b):
    """Host-side index preprocessing. Returns (in_maps, layout) where layout
    gives the compile-time chunk counts per (layer, window), shared by all
    cores."""
    x = np.asarray(x, dtype=np.float32)
    ei = np.asarray(edge_indices).astype(np.int64)
    W = np.asarray(W, dtype=np.float32)
    b = np.asarray(b, dtype=np.float32)

    import ml_dtypes
    BF16 = ml_dtypes.bfloat16
    # per-core constant inputs
    iota_row = np.broadcast_to(
        np.arange(P, dtype=np.float32)[None, :], (P, P)
    ).astype(BF16)
    ident = np.eye(P, dtype=np.float32)
    bb = b.reshape(1, N_LAYERS * DIM).copy()

    xts = []
    for c in range(N_CORES):
        xs = x[c * NSH : (c + 1) * NSH]                      # [12500, 128]
        xp = np.zeros((NSHP, DIM), dtype=np.float32)
        xp[:NSH] = xs
        xts.append(np.ascontiguousarray(xp.T))               # [128, 12544]

    degs = np.ones((N_CORES, N_LAYERS, P, NWIN), dtype=np.float32)
    per_core_edges = [[None] * N_LAYERS for _ in range(N_CORES)]
    n_chunks = np.zeros((N_LAYERS, NWIN), dtype=np.int64)

    for l in range(N_LAYERS):
        row = ei[l, 0]
        col = ei[l, 1]
        deg = np.bincount(col, minlength=N_NODES).astype(np.float32) + 1.0
        src_pad = ((row // NSH) * NSHP + (row % NSH)).astype(np.int32)
        core_of = col // NSH
        lcol = (col % NSH).astype(np.int32)
        win = lcol // P
        dloc = (lcol % P).astype(np.float32)
        for c in range(N_CORES):
            m = core_of == c
            wc, dc, sc = win[m], dloc[m], src_pad[m]
            order = np.argsort(wc, kind="stable")
            wc, dc, sc = wc[order], dc[order], sc[order]
            cnt = np.bincount(wc, minlength=NWIN)
            per_core_edges[c][l] = (cnt, dc, sc)
            dlp = np.ones(NSHP, dtype=np.float32)
            dlp[:NSH] = deg[c * NSH : (c + 1) * NSH]
            degs[c, l] = dlp.reshape(NWIN, P).T
        cnts = np.stack([per_core_edges[c][l][0] for c in range(N_CORES)])
        # self loops are handled in the window epilogue, not as edge chunks
        # slot-stream layout: window w gets exactly max_c cnt slots (no
        # per-window rounding); chunks of 128 slots may span window boundaries
        # align to 64 so chunk-internal window splits land on legal matmul
        # base partitions (0/32/64; 96 is rejected by the PE)
        mx = np.maximum(cnts.max(axis=0), 1)
        n_chunks[l] = (mx + 63) // 64 * 64

    slots_layer = n_chunks.sum(axis=1)
    t_layer = (slots_layer + (P - 1)) // P         # chunks per layer
    tmax = int(t_layer.max())

    srcs_all = np.zeros((N_CORES, N_LAYERS, P, tmax), dtype=np.int32)
    dloc_all = np.full((N_CORES, N_LAYERS, P, tmax), -1.0, dtype=np.float32)
    for l in range(N_LAYERS):
        for c in range(N_CORES):
            cnt, dc, sc = per_core_edges[c][l]
            off = np.concatenate([[0], np.cumsum(cnt)[:-1]])
            tl = int(t_layer[l])
            s_arr = np.zeros((tl * P,), dtype=np.int32)
            d_arr = np.full((tl * P,), -1.0, dtype=np.float32)
            pos = 0
            for w in range(NWIN):
                k = int(n_chunks[l, w])          # slots for window w
                nreal = int(cnt[w])
                s_arr[pos : pos + nreal] = sc[off[w] : off[w] + nreal]
                d_arr[pos : pos + nreal] = dc[off[w] : off[w] + nreal]
                pos += k
            srcs_all[c, l, :, :tl] = s_arr.reshape(tl, P).T
            dloc_all[c, l, :, :tl] = d_arr.reshape(tl, P).T

    in_maps = []
    for c in range(N_CORES):
        in_maps.append(
            {
                "xt": xts[c],
                "wmat": W,
                "bb": bb,
                "iota": iota_row,
                "ident": ident,
                "degs": degs[c],
                "srcs": srcs_all[c],
                "dlocs": dloc_all[c].astype(BF16),
            }
        )
    layout = (n_chunks, t_layer, tmax)
    return in_maps, layout


def _build(layout, skip_collective=False, msg_bufs=16):
    n_chunks, t_layer, tmax = layout
    nc = bacc.Bacc(
        "TRN2", target_bir_lowering=False, debug=False, num_devices=N_CORES
    )
    xt_in = nc.dram_tensor("xt", [P, NSHP], F32, kind="ExternalInput").ap()
    w_in = nc.dram_tensor("wmat", [N_LAYERS, DIM, DIM], F32, kind="ExternalInput").ap()
    b_in = nc.dram_tensor("bb", [1, N_LAYERS * DIM], F32, kind="ExternalInput").ap()
    iota_in = nc.dram_tensor("iota", [P, P], BF, kind="ExternalInput").ap()
    id_in = nc.dram_tensor("ident", [P, P], F32, kind="ExternalInput").ap()
    deg_in = nc.dram_tensor("degs", [N_LAYERS, P, NWIN], F32, kind="ExternalInput").ap()
    srcs_in = nc.dram_tensor("srcs", [N_LAYERS, P, tmax], I32, kind="ExternalInput").ap()
    dloc_in = nc.dram_tensor("dlocs", [N_LAYERS, P, tmax], BF, kind="ExternalInput").ap()
    out_ap = nc.dram_tensor("out", [NSHP, DIM], F32, kind="ExternalOutput").ap()

    hloc16 = nc.dram_tensor("hloc16", [NSHP, DIM], BF).ap()
    table16 = nc.dram_tensor("table16", [N_TABLE, DIM], BF, addr_space="Shared").ap()
    hloc8 = nc.dram_tensor("hloc8", [NSHP, DIM], F8).ap()
    table8 = nc.dram_tensor("table8", [N_TABLE, DIM], F8, addr_space="Shared").ap()

    with tile.TileContext(nc) as tc:
        with (
            tc.tile_pool(name="const", bufs=1) as constp,
            tc.tile_pool(name="xt", bufs=1) as xtp,
            tc.tile_pool(name="edges", bufs=2) as edgep,
            tc.tile_pool(name="msg", bufs=msg_bufs) as msgp,
            tc.tile_pool(name="sel", bufs=2) as selp,
            tc.tile_pool(name="hsb", bufs=1) as hsbp,
            tc.tile_pool(name="tr", bufs=3) as trp,
            tc.tile_pool(name="ph", bufs=2, space="PSUM") as php,
            tc.tile_pool(name="pw", bufs=2, space="PSUM") as pwp,
            tc.tile_pool(name="pt", bufs=2, space="PSUM") as ptp,
            tc.tile_pool(name="pb", bufs=1, space="PSUM") as pbp,
        ):
            # constants
            iota_sb = constp.tile([P, P], BF)
            nc.sync.dma_start(out=iota_sb[:], in_=iota_in[:])
            ident_sb = constp.tile([P, P], F32)
            nc.sync.dma_start(out=ident_sb[:], in_=id_in[:])
            w_sb = constp.tile([P, N_LAYERS * DIM], F32)
            for l in range(N_LAYERS):
                nc.sync.dma_start(
                    out=w_sb[:, l * DIM : (l + 1) * DIM], in_=w_in[l]
                )
            brow_sb = constp.tile([1, N_LAYERS * DIM], F32)
            nc.sync.dma_start(out=brow_sb[:], in_=b_in[:])
            ones_row = constp.tile([1, P], F32)
            nc.vector.memset(ones_row[:], 1.0)

            xt_sb = xtp.tile([P, NSHP], F32)
            nc.sync.dma_start(out=xt_sb[:], in_=xt_in[:])

            state = {}

            def epilogue(l, w, pw):
                dis_sb = state["dis_sb"]
                bbc_sb = state["bbc_sb"]
                hsb = state["hsb"]
                t0g = trp.tile([P, P], F32, tag="t0g")
                nc.vector.tensor_tensor(
                    out=t0g[:],
                    in0=pw[:],
                    in1=hsb[:, w * P : (w + 1) * P],
                    op=mybir.AluOpType.add,
                )
                t1 = trp.tile([P, P], F32, tag="t1")
                nc.vector.tensor_scalar(
                    out=t1[:],
                    in0=t0g[:],
                    scalar1=dis_sb[:, w : w + 1],
                    scalar2=None,
                    op0=mybir.AluOpType.mult,
                )
                nc.vector.tensor_tensor(
                    out=t1[:], in0=t1[:], in1=bbc_sb[:], op=mybir.AluOpType.add
                )
                t2 = trp.tile([P, P], F32, tag="t2")
                nc.vector.tensor_scalar(
                    out=t2[:],
                    in0=t1[:],
                    scalar1=0.0,
                    scalar2=None,
                    op0=mybir.AluOpType.max,
                )
                if l < N_LAYERS - 1:
                    pt = ptp.tile([P, P], F32, space="PSUM", tag="pt")
                    nc.tensor.transpose(out=pt[:], in_=t2[:], identity=ident_sb[:])
                    nc.vector.tensor_copy(
                        out=state["xt_sb"][:, w * P : (w + 1) * P], in_=pt[:]
                    )
                else:
                    nc.sync.dma_start(
                        out=out_ap[w * P : (w + 1) * P, :], in_=t2[:]
                    )

            state["xt_sb"] = xt_sb
            for l in range(N_LAYERS):
                tl = int(t_layer[l])
                # --- normalization coefficients ---
                deg_sb = trp.tile([P, NWIN], F32, tag="deg")
                nc.sync.dma_start(out=deg_sb[:], in_=deg_in[l])
                dis_sb = trp.tile([P, NWIN], F32, tag="dis")
                nc.vector.reciprocal(dis_sb[:], deg_sb[:])
                nc.scalar.activation(
                    dis_sb[:], dis_sb[:], mybir.ActivationFunctionType.Sqrt
                )

                # --- b broadcast tile: ones_row^T (x) b_row ---
                pb = pbp.tile([P, P], F32, space="PSUM", tag="pb")
                nc.tensor.matmul(
                    out=pb[:],
                    lhsT=ones_row[:],
                    rhs=brow_sb[:, l * DIM : (l + 1) * DIM],
                    start=True,
                    stop=True,
                )
                bbc_sb = trp.tile([P, P], F32, tag="bbc")
                nc.vector.tensor_copy(out=bbc_sb[:], in_=pb[:])
                state["dis_sb"] = dis_sb
                state["bbc_sb"] = bbc_sb

                # --- H stage: table_local = dis * (X @ W) ---
                fp8 = FP8_LAYERS[l]
                mdt = F8 if fp8 else BF
                hloc, table = (hloc8, table8) if fp8 else (hloc16, table16)
                hsb = hsbp.tile([P, NWIN * P], mdt, tag="hsb8" if fp8 else "hsb")
                state["hsb"] = hsb
                for w in range(NWIN):
                    ph = php.tile([P, P], F32, space="PSUM", tag="ph")
                    nc.tensor.matmul(
                        out=ph[:],
                        lhsT=xt_sb[:, w * P : (w + 1) * P],
                        rhs=w_sb[:, l * DIM : (l + 1) * DIM],
                        start=True,
                        stop=True,
                    )
                    nc.vector.tensor_scalar(
                        out=hsb[:, w * P : (w + 1) * P],
                        in0=ph[:],
                        scalar1=dis_sb[:, w : w + 1],
                        scalar2=None,
                        op0=mybir.AluOpType.mult,
                    )
                nc.sync.dma_start(
                    out=hloc[:].rearrange("(w p) f -> p w f", p=P),
                    in_=hsb[:].rearrange("p (w f) -> p w f", f=DIM),
                )

                # --- replicate table ---
                if skip_collective:
                    nc.sync.dma_start(out=table[:NSHP, :], in_=hloc[:])
                else:
                    nc.gpsimd.collective_compute(
                        "AllGather",
                        mybir.AluOpType.bypass,
                        replica_groups=[list(range(N_CORES))],
                        ins=[hloc[:]],
                        outs=[table[:]],
                    )

                # --- edge metadata for this layer ---
                srcs_sb = edgep.tile([P, tl], I32, tag="srcs")
                nc.sync.dma_start(out=srcs_sb[:], in_=srcs_in[l, :, :tl])
                dloc_sb = edgep.tile([P, tl], BF, tag="dlocs")
                nc.sync.dma_start(out=dloc_sb[:], in_=dloc_in[l, :, :tl])

                # --- scatter stage (slot-stream chunking) ---
                # window w owns slot range [wstart[w], wstart[w+1]); chunks are
                # 128-slot groups; a chunk may span window boundaries and is
                # consumed by per-window matmuls over partition subranges.
                slots = [int(n_chunks[l, w]) for w in range(NWIN)]
                wstart = [0]
                for w in range(NWIN):
                    wstart.append(wstart[-1] + slots[w])
                total_slots = wstart[-1]
                SB = 8  # chunks per S-build batch
                pw = None
                first = True
                wptr = 0
                s_base = 0
                for t in range(tl):
                    if t % SB == 0:
                        nb = min(SB, tl - t)
                        s_sb = selp.tile([P, SB * P], BF, tag="sel")
                        nc.vector.tensor_tensor(
                            out=s_sb[:, : nb * P].rearrange(
                                "p (k j) -> p k j", k=nb
                            ),
                            in0=dloc_sb[:, t : t + nb]
                            .unsqueeze(2)
                            .to_broadcast([P, nb, P]),
                            in1=iota_sb[:]
                            .unsqueeze(1)
                            .to_broadcast([P, nb, P]),
                            op=mybir.AluOpType.is_equal,
                        )
                        s_base = t
                    msg = msgp.tile([P, P], mdt, tag="msg8" if fp8 else "msg")
                    nc.gpsimd.indirect_dma_start(
                        out=msg[:],
                        out_offset=None,
                        in_=table[:],
                        in_offset=bass.IndirectOffsetOnAxis(
                            ap=srcs_sb[:, t : t + 1], axis=0
                        ),
                    )
                    scol = (t - s_base) * P
                    lo = t * P
                    hi = min(lo + P, total_slots)
                    a = 0
                    while lo + a < hi:
                        while wstart[wptr + 1] <= lo + a:
                            wptr += 1
                        w = wptr
                        bnd = min(hi, wstart[w + 1]) - lo
                        if pw is None:
                            pw = pwp.tile([P, P], F32, space="PSUM", tag="pw")
                            first = True
                        is_last = lo + bnd == wstart[w + 1]
                        nc.tensor.matmul(
                            out=pw[:],
                            lhsT=s_sb[a:bnd, scol : scol + P],
                            rhs=msg[a:bnd, :],
                            start=first,
                            stop=is_last,
                        )
                        first = False
                        if is_last:
                            epilogue(l, w, pw)
                            pw = None
                        a = bnd

    nc.compile()
    return nc


def build_all(x, edge_indices, W, b):
    in_maps, layout = _prepare(x, edge_indices, W, b)
    nc = _build(layout)
    return nc, in_maps


def kernel(x, edge_indices, W, b):
    nc, in_maps = build_all(x, edge_indices, W, b)
    last_err = None
    for _ in range(3):  # retry transient NRT/axon device faults
        try:
            res = run_bass_kernel_spmd(nc, in_maps, list(range(N_CORES)))
            break
        except Exception as e:  # noqa: BLE001
            last_err = e
            import time as _time

            _time.sleep(5.0)
    else:
        raise last_err
    out = np.concatenate(
        [res.results[c]["out"][:NSH] for c in range(N_CORES)], axis=0
    )
    return out.astype(np.float32)

